# revision 22
# baseline (speedup 1.0000x reference)
"""Trainium2 Bass kernel for: Conv3d(3->16, k=3x3x3, VALID) + bias -> min over
depth -> softmax over channels.

Input  x: (16, 3, 32, 128, 128) f32   [N, C_in, D, H, W]
Weight w: (16, 3, 3, 3, 3) f32        [C_out, C_in, kD, kH, kW]
Bias   b: (16,) f32
Output  : (16, 16, 126, 126) f32      [N, C_out, H_out, W_out]

Data-parallel over batch: 2 batches per core x 8 cores. Per core:

  - x stored per (batch, h-half) as one [128, 8768] bf16 tile: strip r
    (partition quadrant 32r) holds 30 rows = (10 input depths 8r..8r+9) x
    (ci 3); free dim = local (h, w) flattened (66 or 64 h-rows + pad).
  - Conv as 4 row-packed matmuls per (chunk, khw): tile r = [K<=30, M=128,
    N=512] at tile_position (32r, 0); M = 8 local douts x 16 co; 9
    accumulating MMs over (kh,kw) with free-dim-shifted rhs (stride-1 conv
    == same spatial layout + offset koff). Weight col block[(dl,ci),
    (dll,co)] = w[co,ci,dl-dll,kh,kw]. PSUM supertile [128, 4*512]:
    bank r = strip r's 8 douts.
  - Strip 3 douts 30,31 don't exist: their weight cols are 0 except a
    BIG=32768 entry at khw=0 against a constant-1.0 rhs row, so those psum
    lanes hold +32768 and never win the depth-min.
  - Depth-min: one DVE tensor_reduce(min) over the 4 banks -> [128, 512],
    then a 128->64->32->16 partition tree-min on GpSimd into a [128, 512]
    collector (8 chunks x 16 co).
  - Softmax over co per 8-chunk group: ACT exp (bias fused; min(y)+b ==
    min(y+b)), PE ones-matmul for co-sums, DVE reciprocal, PE broadcast
    matmul, DVE multiply, DMA out valid rows.
"""

import os
import sys

sys.path.insert(0, "/opt/trn_rl_repo")

import numpy as np
import ml_dtypes

import concourse.bass as bass
import concourse.bacc as bacc
import concourse.tile as tile
import concourse.mybir as mybir
import concourse.bass_isa as bass_isa
from concourse import bass_utils

F32 = mybir.dt.float32
BF16 = mybir.dt.bfloat16

N_CORES = 8
NB = 2           # batches per core
CI = 3
D = 32
H = 128
W = 128
CO = 16
CHUNK = 512
HOUT = 126
WOUT = 126
PAD = 320
QF = 66 * W + PAD  # quad tile free size (worst case hh=0)
BIG = 32768.0

_COMPILED = {}


def _strip_depths(r):
    return 10 if r < 3 else 8


def _kr(r, khw):
    if r < 3:
        return 30
    return 25 if khw == 0 else 24


def _build_weight_blocks(conv_weight):
    """[128, 9*128]: strip r rows 32r+(3*dl+ci); col khw*128 + dll*16 + co
    = w[co, ci, dl-dll, kh, kw] (0 outside kd range / dout>=30). Row 120
    (strip 3 local 24) carries BIG at khw=0 for the dout 30/31 lanes."""
    wb = np.zeros((128, 9 * 128), dtype=np.float32)
    for r in range(4):
        for dl in range(_strip_depths(r)):
            for ci in range(CI):
                row = 32 * r + 3 * dl + ci
                for khw in range(9):
                    kh, kw = khw // 3, khw % 3
                    for dll in range(8):
                        kd = dl - dll
                        if 8 * r + dll < 30 and 0 <= kd <= 2:
                            wb[row, khw * 128 + dll * 16:
                               khw * 128 + dll * 16 + 16] = \
                                conv_weight[:, ci, kd, kh, kw]
    for dll in (6, 7):
        wb[120, dll * 16:dll * 16 + 16] = BIG
    return wb.astype(ml_dtypes.bfloat16)


def _build_ones():
    """[128, 8]: col j sums partitions {16j + co : co}."""
    ones = np.zeros((128, 8), dtype=np.float32)
    for p in range(128):
        ones[p, p // 16] = 1.0
    return ones


def _build_ones_bc():
    """[8, 128]: transpose -- broadcasts row j over its 16-partition group."""
    return np.ascontiguousarray(_build_ones().T)


def _build_bias128(conv_bias):
    """[128, 1]: partition 16j + co -> bias[co]."""
    b = np.zeros((128, 1), dtype=np.float32)
    for p in range(128):
        b[p, 0] = conv_bias[p % 16]
    return b


def _emit_kernel(tc):
    nc = tc.nc
    x_ap = nc.dram_tensor("x", [NB, D, CI, H, W], BF16,
                          kind="ExternalInput").ap()
    w_ap = nc.dram_tensor("w", [128, 9 * 128], BF16,
                          kind="ExternalInput").ap()
    bias_ap = nc.dram_tensor("bias", [128, 1], F32, kind="ExternalInput").ap()
    ones_ap = nc.dram_tensor("ones", [128, 8], F32, kind="ExternalInput").ap()
    ones_bc_ap = nc.dram_tensor("ones_bc", [8, 128], F32,
                                kind="ExternalInput").ap()
    y_ap = nc.dram_tensor("y", [NB, CO, HOUT, WOUT], F32,
                          kind="ExternalOutput").ap()

    from contextlib import ExitStack

    with ExitStack() as ctx:
        const_pool = ctx.enter_context(tc.tile_pool(name="const", bufs=1))
        in_pool = ctx.enter_context(tc.tile_pool(name="in", bufs=2))
        m4w_pool = ctx.enter_context(tc.tile_pool(name="m4w", bufs=2))
        red_pool = ctx.enter_context(tc.tile_pool(name="red", bufs=2))
        sm_pool = ctx.enter_context(tc.tile_pool(name="sm", bufs=2))
        psum_pool = ctx.enter_context(tc.tile_pool(name="ps", bufs=4,
                                                   space="PSUM"))

        w_sb = const_pool.tile([128, 9 * 128], BF16, tag="w")
        nc.sync.dma_start(w_sb[:, :], w_ap[:, :])
        bias_sb = const_pool.tile([128, 1], F32, tag="bias")
        nc.sync.dma_start(bias_sb[:, :], bias_ap[:, :])
        ones_sb = const_pool.tile([128, 8], F32, tag="ones")
        nc.sync.dma_start(ones_sb[:, :], ones_ap[:, :])
        ones_bc_sb = const_pool.tile([8, 128], F32, tag="onesbc")
        nc.sync.dma_start(ones_bc_sb[:, :], ones_bc_ap[:, :])

        def emit_tail_a(t):
            m4w_t, n_t, h0_t, q_t, st = t
            # fold the 8 dout_local groups (partition tree-min); engine TTs
            # need equal SBUF partition bases, so shift the upper half down
            # via SBUF-to-SBUF DMA at each level.
            sh = sm_pool.tile([64, 8 * CHUNK], BF16, tag="sh")
            nc.sync.dma_start(sh[0:64, :], m4w_t[64:128, :])
            nc.vector.tensor_tensor(
                out=m4w_t[0:64, :], in0=m4w_t[0:64, :], in1=sh[0:64, :],
                op=mybir.AluOpType.min)
            nc.sync.dma_start(sh[0:32, :], m4w_t[32:64, :])
            nc.vector.tensor_tensor(
                out=m4w_t[0:32, :], in0=m4w_t[0:32, :], in1=sh[0:32, :],
                op=mybir.AluOpType.min)
            nc.sync.dma_start(sh[0:16, :], m4w_t[16:32, :])
            nc.vector.tensor_tensor(
                out=m4w_t[0:16, :], in0=m4w_t[0:16, :], in1=sh[0:16, :],
                op=mybir.AluOpType.min)
            # spread mins to the (chunk x co) softmax layout
            coll = sm_pool.tile([128, CHUNK], BF16, tag="coll")
            for j in range(8):
                nc.sync.dma_start(coll[16 * j:16 * j + 16, :],
                                  m4w_t[0:16, j * CHUNK:(j + 1) * CHUNK])
            expt = sm_pool.tile([128, CHUNK], F32, tag="exp")
            nc.scalar.activation(
                expt[:, :], coll[:, :],
                mybir.ActivationFunctionType.Exp,
                bias=bias_sb[:, :], scale=1.0)
            st["expt"] = expt

        def emit_tail_b(t):
            m4w_t, n_t, h0_t, q_t, st = t
            # esum and rb share one psum tile: esum lives in partitions 0..8,
            # is consumed by the reciprocal, then rb's matmul overwrites.
            T = psum_pool.tile([128, CHUNK], F32, tag="big")
            nc.tensor.matmul(T[0:8, :], lhsT=ones_sb[:, :],
                             rhs=st["expt"][:, :], start=True, stop=True)
            rec = sm_pool.tile([8, CHUNK], F32, tag="rec")
            nc.vector.reciprocal_approx_fast(rec[:, :], T[0:8, :])
            st["rec"] = rec
            st["T"] = T

        def emit_tail_c(t):
            m4w_t, n_t, h0_t, q_t, st = t
            T = st["T"]
            nc.tensor.matmul(T[:, :], lhsT=ones_bc_sb[:, :],
                             rhs=st["rec"][:, :], start=True, stop=True)
            soft = sm_pool.tile([128, CHUNK], F32, tag="soft")
            nc.vector.tensor_tensor(
                out=soft[:, :], in0=T[:, :], in1=st["expt"][:, :],
                op=mybir.AluOpType.mult)
            for j in range(8):
                hc = h0_t + 4 * (8 * q_t + j)
                ni = min(4, HOUT - hc)
                if ni <= 0:
                    continue
                srcj = soft[16 * j:16 * j + 16, :].rearrange(
                    "co (i w) -> co i w", i=4)[:, 0:ni, 0:WOUT]
                nc.sync.dma_start(y_ap[n_t, :, hc:hc + ni, :], srcj)

        pending = None
        items = [(n, hh) for n in range(NB) for hh in range(2)]
        quads = {}

        def prep_quad(k):
            n_k, hh_k = items[k]
            h0_k = 64 * hh_k
            hrows = 66 if hh_k == 0 else 64
            quad_k = in_pool.tile([128, QF], BF16, tag="quad")
            nc.gpsimd.memset(quad_k[:, hrows * W:QF], 0.0)
            # rows 96..127 <- 1.0; strip-3 DMA overwrites 96..119, leaving
            # the constant-1.0 row 120 for the BIG sentinel matmul column.
            nc.gpsimd.memset(quad_k[96:128, 0:hrows * W], 1.0)
            for r in range(4):
                nd = _strip_depths(r)
                srcr = x_ap[n_k, 8 * r:8 * r + nd, :,
                            h0_k:h0_k + hrows, :].rearrange(
                    "d c h w -> (d c) (h w)")
                nc.sync.dma_start(
                    quad_k[32 * r:32 * r + 3 * nd, 0:hrows * W], srcr)
            quads[k] = quad_k

        prep_quad(0)
        for k, (n, hh) in enumerate(items):
            h0 = 64 * hh
            quad = quads.pop(k)
            if True:
                for q in range(2):
                    m4w = m4w_pool.tile([128, 8 * CHUNK], BF16, tag="m4w")
                    for j in range(8):
                        m = 8 * q + j
                        s0 = CHUNK * m
                        psA = psum_pool.tile([128, 2 * CHUNK], F32,
                                             tag="big")
                        psB = psum_pool.tile([128, 2 * CHUNK], F32,
                                             tag="big")
                        for khw in range(9):
                            kh, kw = khw // 3, khw % 3
                            koff = kh * W + kw
                            for r in range(4):
                                kr = _kr(r, khw)
                                pst = psA if r < 2 else psB
                                nc.tensor.matmul(
                                    pst[:, (r % 2) * CHUNK:
                                        (r % 2 + 1) * CHUNK],
                                    lhsT=w_sb[32 * r:32 * r + kr,
                                              khw * 128:(khw + 1) * 128],
                                    rhs=quad[32 * r:32 * r + kr,
                                             s0 + koff:s0 + koff + CHUNK],
                                    start=(khw == 0),
                                    stop=(khw == 8),
                                    tile_position=(32 * r, 0),
                                    skip_group_check=True,
                                )
                        rA = red_pool.tile([128, CHUNK], BF16, tag="rA")
                        nc.vector.tensor_reduce(
                            out=rA[:, :],
                            in_=psA[:, :].rearrange("p (r s) -> p s r",
                                                    s=CHUNK),
                            axis=mybir.AxisListType.X,
                            op=mybir.AluOpType.min,
                        )
                        rB = red_pool.tile([128, CHUNK], BF16, tag="rB")
                        nc.vector.tensor_reduce(
                            out=rB[:, :],
                            in_=psB[:, :].rearrange("p (r s) -> p s r",
                                                    s=CHUNK),
                            axis=mybir.AxisListType.X,
                            op=mybir.AluOpType.min,
                        )
                        nc.vector.tensor_tensor(
                            out=m4w[:, j * CHUNK:(j + 1) * CHUNK],
                            in0=rA[:, :], in1=rB[:, :],
                            op=mybir.AluOpType.min)
                        # software-pipelined: previous group's tail ops
                        # land between this group's chunks so every engine
                        # FIFO sees them with ready inputs and the psum
                        # rotation never blocks on a long chain.
                        if pending is not None:
                            if j == 0:
                                emit_tail_a(pending)
                            elif j == 6:
                                emit_tail_b(pending)
                            elif j == 7:
                                emit_tail_c(pending)
                                pending = None
                        if q == 0 and j == 3 and k + 1 < len(items):
                            prep_quad(k + 1)
                    pending = (m4w, n, h0, q, {})
        emit_tail_a(pending)
        emit_tail_b(pending)
        emit_tail_c(pending)


def _compile():
    if "nc" in _COMPILED:
        return _COMPILED["nc"]
    nc = bacc.Bacc("TRN2", target_bir_lowering=False, debug=False,
                   num_devices=N_CORES)
    with tile.TileContext(nc) as tc:
        _emit_kernel(tc)
    nc.compile()
    _COMPILED["nc"] = nc
    return nc


def kernel(x, conv_weight, conv_bias):
    x = np.asarray(x, dtype=np.float32)
    conv_weight = np.asarray(conv_weight, dtype=np.float32)
    conv_bias = np.asarray(conv_bias, dtype=np.float32)

    xp = np.ascontiguousarray(
        x.transpose(0, 2, 1, 3, 4)).astype(ml_dtypes.bfloat16)  # [N,D,C,H,W]
    w_sb = _build_weight_blocks(conv_weight)
    bias_sb = _build_bias128(conv_bias)
    ones_sb = _build_ones()
    ones_bc_sb = _build_ones_bc()

    nc = _compile()
    in_maps = []
    for i in range(N_CORES):
        in_maps.append({
            "x": np.ascontiguousarray(xp[NB * i:NB * (i + 1)]),
            "w": w_sb,
            "bias": bias_sb,
            "ones": ones_sb,
            "ones_bc": ones_bc_sb,
        })
    res = bass_utils.run_bass_kernel_spmd(
        nc, in_maps, core_ids=list(range(N_CORES)),
        trace=bool(int(os.environ.get("KERNEL_TRACE", "0"))),
    )
    _COMPILED["last_results"] = res
    out = np.concatenate([res.results[i]["y"] for i in range(N_CORES)], axis=0)
    return out


if __name__ == "__main__":
    _compile()
    print("build OK")


# revision 23
# speedup vs baseline: 1.0337x; 1.0337x over previous
"""Trainium2 Bass kernel for: Conv3d(3->16, k=3x3x3, VALID) + bias -> min over
depth -> softmax over channels.

Input  x: (16, 3, 32, 128, 128) f32   [N, C_in, D, H, W]
Weight w: (16, 3, 3, 3, 3) f32        [C_out, C_in, kD, kH, kW]
Bias   b: (16,) f32
Output  : (16, 16, 126, 126) f32      [N, C_out, H_out, W_out]

Data-parallel over batch: 2 batches per core x 8 cores. Per core:

  - x stored per (batch, h-half) as one [128, 8768] bf16 tile: strip r
    (partition quadrant 32r) holds 30 rows = (10 input depths 8r..8r+9) x
    (ci 3); free dim = local (h, w) flattened (66 or 64 h-rows + pad).
  - Conv as 4 row-packed matmuls per (chunk, khw): tile r = [K<=30, M=128,
    N=512] at tile_position (32r, 0); M = 8 local douts x 16 co; 9
    accumulating MMs over (kh,kw) with free-dim-shifted rhs (stride-1 conv
    == same spatial layout + offset koff). Weight col block[(dl,ci),
    (dll,co)] = w[co,ci,dl-dll,kh,kw]. PSUM supertile [128, 4*512]:
    bank r = strip r's 8 douts.
  - Strip 3 douts 30,31 don't exist: their weight cols are 0 except a
    BIG=32768 entry at khw=0 against a constant-1.0 rhs row, so those psum
    lanes hold +32768 and never win the depth-min.
  - Depth-min: one DVE tensor_reduce(min) over the 4 banks -> [128, 512],
    then a 128->64->32->16 partition tree-min on GpSimd into a [128, 512]
    collector (8 chunks x 16 co).
  - Softmax over co per 8-chunk group: ACT exp (bias fused; min(y)+b ==
    min(y+b)), PE ones-matmul for co-sums, DVE reciprocal, PE broadcast
    matmul, DVE multiply, DMA out valid rows.
"""

import os
import sys

sys.path.insert(0, "/opt/trn_rl_repo")

import numpy as np
import ml_dtypes

import concourse.bass as bass
import concourse.bacc as bacc
import concourse.tile as tile
import concourse.mybir as mybir
import concourse.bass_isa as bass_isa
from concourse import bass_utils

F32 = mybir.dt.float32
BF16 = mybir.dt.bfloat16

N_CORES = 8
NB = 2           # batches per core
CI = 3
D = 32
H = 128
W = 128
CO = 16
CHUNK = 512
HOUT = 126
WOUT = 126
PAD = 320
QF = 66 * W + PAD  # quad tile free size (worst case hh=0)
BIG = 32768.0

_COMPILED = {}


def _strip_depths(r):
    return 10 if r < 3 else 8


def _kr(r, khw):
    if r < 3:
        return 30
    return 25 if khw == 0 else 24


def _build_weight_blocks(conv_weight):
    """[128, 9*128]: strip r rows 32r+(3*dl+ci); col khw*128 + dll*16 + co
    = w[co, ci, dl-dll, kh, kw] (0 outside kd range / dout>=30). Row 120
    (strip 3 local 24) carries BIG at khw=0 for the dout 30/31 lanes."""
    wb = np.zeros((128, 9 * 128), dtype=np.float32)
    for r in range(4):
        for dl in range(_strip_depths(r)):
            for ci in range(CI):
                row = 32 * r + 3 * dl + ci
                for khw in range(9):
                    kh, kw = khw // 3, khw % 3
                    for dll in range(8):
                        kd = dl - dll
                        if 8 * r + dll < 30 and 0 <= kd <= 2:
                            wb[row, khw * 128 + dll * 16:
                               khw * 128 + dll * 16 + 16] = \
                                conv_weight[:, ci, kd, kh, kw]
    for dll in (6, 7):
        wb[120, dll * 16:dll * 16 + 16] = BIG
    return wb.astype(ml_dtypes.bfloat16)


def _build_ones():
    """[128, 8]: col j sums partitions {16j + co : co}."""
    ones = np.zeros((128, 8), dtype=np.float32)
    for p in range(128):
        ones[p, p // 16] = 1.0
    return ones


def _build_ones_bc():
    """[8, 128]: transpose -- broadcasts row j over its 16-partition group."""
    return np.ascontiguousarray(_build_ones().T)


def _build_bias128(conv_bias):
    """[128, 1]: partition 16j + co -> bias[co]."""
    b = np.zeros((128, 1), dtype=np.float32)
    for p in range(128):
        b[p, 0] = conv_bias[p % 16]
    return b


def _emit_kernel(tc):
    nc = tc.nc
    x_ap = nc.dram_tensor("x", [NB, D, CI, H, W], BF16,
                          kind="ExternalInput").ap()
    w_ap = nc.dram_tensor("w", [128, 9 * 128], BF16,
                          kind="ExternalInput").ap()
    bias_ap = nc.dram_tensor("bias", [128, 1], F32, kind="ExternalInput").ap()
    ones_ap = nc.dram_tensor("ones", [128, 8], F32, kind="ExternalInput").ap()
    ones_bc_ap = nc.dram_tensor("ones_bc", [8, 128], F32,
                                kind="ExternalInput").ap()
    y_ap = nc.dram_tensor("y", [NB, CO, HOUT, WOUT], F32,
                          kind="ExternalOutput").ap()

    from contextlib import ExitStack

    with ExitStack() as ctx:
        const_pool = ctx.enter_context(tc.tile_pool(name="const", bufs=1))
        in_pool = ctx.enter_context(tc.tile_pool(name="in", bufs=2))
        m4w_pool = ctx.enter_context(tc.tile_pool(name="m4w", bufs=2))
        sm_pool = ctx.enter_context(tc.tile_pool(name="sm", bufs=2))
        psum_pool = ctx.enter_context(tc.tile_pool(name="ps", bufs=2,
                                                   space="PSUM"))

        w_sb = const_pool.tile([128, 9 * 128], BF16, tag="w")
        nc.sync.dma_start(w_sb[:, :], w_ap[:, :])
        bias_sb = const_pool.tile([128, 1], F32, tag="bias")
        nc.sync.dma_start(bias_sb[:, :], bias_ap[:, :])
        ones_sb = const_pool.tile([128, 8], F32, tag="ones")
        nc.sync.dma_start(ones_sb[:, :], ones_ap[:, :])
        ones_bc_sb = const_pool.tile([8, 128], F32, tag="onesbc")
        nc.sync.dma_start(ones_bc_sb[:, :], ones_bc_ap[:, :])

        def emit_tail_a(t):
            m4w_t, n_t, h0_t, q_t, st = t
            # fold the 8 dout_local groups (partition tree-min); engine TTs
            # need equal SBUF partition bases, so shift the upper half down
            # via SBUF-to-SBUF DMA at each level.
            sh = sm_pool.tile([64, 8 * CHUNK], BF16, tag="sh")
            nc.sync.dma_start(sh[0:64, :], m4w_t[64:128, :])
            nc.vector.tensor_tensor(
                out=m4w_t[0:64, :], in0=m4w_t[0:64, :], in1=sh[0:64, :],
                op=mybir.AluOpType.min)
            nc.sync.dma_start(sh[0:32, :], m4w_t[32:64, :])
            nc.vector.tensor_tensor(
                out=m4w_t[0:32, :], in0=m4w_t[0:32, :], in1=sh[0:32, :],
                op=mybir.AluOpType.min)
            nc.sync.dma_start(sh[0:16, :], m4w_t[16:32, :])
            nc.vector.tensor_tensor(
                out=m4w_t[0:16, :], in0=m4w_t[0:16, :], in1=sh[0:16, :],
                op=mybir.AluOpType.min)
            # spread mins to the (chunk x co) softmax layout
            coll = sm_pool.tile([128, CHUNK], BF16, tag="coll")
            for j in range(8):
                nc.sync.dma_start(coll[16 * j:16 * j + 16, :],
                                  m4w_t[0:16, j * CHUNK:(j + 1) * CHUNK])
            expt = sm_pool.tile([128, CHUNK], F32, tag="exp")
            nc.scalar.activation(
                expt[:, :], coll[:, :],
                mybir.ActivationFunctionType.Exp,
                bias=bias_sb[:, :], scale=1.0)
            st["expt"] = expt

        def emit_tail_b(t):
            m4w_t, n_t, h0_t, q_t, st = t
            # esum and rb share one psum tile: esum lives in partitions 0..8,
            # is consumed by the reciprocal, then rb's matmul overwrites.
            T = psum_pool.tile([128, CHUNK], F32, tag="big")
            nc.tensor.matmul(T[0:8, :], lhsT=ones_sb[:, :],
                             rhs=st["expt"][:, :], start=True, stop=True)
            rec = sm_pool.tile([8, CHUNK], F32, tag="rec")
            nc.vector.reciprocal_approx_fast(rec[:, :], T[0:8, :])
            st["rec"] = rec
            st["T"] = T

        def emit_tail_c(t):
            m4w_t, n_t, h0_t, q_t, st = t
            T = st["T"]
            nc.tensor.matmul(T[:, :], lhsT=ones_bc_sb[:, :],
                             rhs=st["rec"][:, :], start=True, stop=True)
            soft = sm_pool.tile([128, CHUNK], F32, tag="soft")
            nc.vector.tensor_tensor(
                out=soft[:, :], in0=T[:, :], in1=st["expt"][:, :],
                op=mybir.AluOpType.mult)
            for j in range(8):
                hc = h0_t + 4 * (8 * q_t + j)
                ni = min(4, HOUT - hc)
                if ni <= 0:
                    continue
                srcj = soft[16 * j:16 * j + 16, :].rearrange(
                    "co (i w) -> co i w", i=4)[:, 0:ni, 0:WOUT]
                nc.sync.dma_start(y_ap[n_t, :, hc:hc + ni, :], srcj)

        pending = None
        items = [(n, hh) for n in range(NB) for hh in range(2)]
        quads = {}

        def prep_quad(k):
            n_k, hh_k = items[k]
            h0_k = 64 * hh_k
            hrows = 66 if hh_k == 0 else 64
            quad_k = in_pool.tile([128, QF], BF16, tag="quad")
            nc.gpsimd.memset(quad_k[:, hrows * W:QF], 0.0)
            # rows 96..127 <- 1.0; strip-3 DMA overwrites 96..119, leaving
            # the constant-1.0 row 120 for the BIG sentinel matmul column.
            nc.gpsimd.memset(quad_k[96:128, 0:hrows * W], 1.0)
            for r in range(4):
                nd = _strip_depths(r)
                srcr = x_ap[n_k, 8 * r:8 * r + nd, :,
                            h0_k:h0_k + hrows, :].rearrange(
                    "d c h w -> (d c) (h w)")
                nc.sync.dma_start(
                    quad_k[32 * r:32 * r + 3 * nd, 0:hrows * W], srcr)
            quads[k] = quad_k

        prep_quad(0)
        for k, (n, hh) in enumerate(items):
            h0 = 64 * hh
            quad = quads.pop(k)
            if True:
                for q in range(2):
                    m4w = m4w_pool.tile([128, 8 * CHUNK], BF16, tag="m4w")
                    for j in range(8):
                        m = 8 * q + j
                        s0 = CHUNK * m
                        ps = psum_pool.tile([128, 4 * CHUNK], F32, tag="big")
                        for khw in range(9):
                            kh, kw = khw // 3, khw % 3
                            koff = kh * W + kw
                            for r in range(4):
                                kr = _kr(r, khw)
                                nc.tensor.matmul(
                                    ps[:, r * CHUNK:(r + 1) * CHUNK],
                                    lhsT=w_sb[32 * r:32 * r + kr,
                                              khw * 128:(khw + 1) * 128],
                                    rhs=quad[32 * r:32 * r + kr,
                                             s0 + koff:s0 + koff + CHUNK],
                                    start=(khw == 0),
                                    stop=(khw == 8),
                                    tile_position=(32 * r, 0),
                                    skip_group_check=True,
                                )
                        nc.vector.tensor_reduce(
                            out=m4w[:, j * CHUNK:(j + 1) * CHUNK],
                            in_=ps[:, :].rearrange("p (r s) -> p s r",
                                                   s=CHUNK),
                            axis=mybir.AxisListType.X,
                            op=mybir.AluOpType.min,
                        )
                        # software-pipelined: previous group's tail ops
                        # land between this group's chunks so every engine
                        # FIFO sees them with ready inputs and the psum
                        # rotation never blocks on a long chain.
                        if pending is not None:
                            if j == 0:
                                emit_tail_a(pending)
                            elif j == 5:
                                emit_tail_b(pending)
                            elif j == 7:
                                emit_tail_c(pending)
                                pending = None
                        if q == 0 and j == 3 and k + 1 < len(items):
                            prep_quad(k + 1)
                    pending = (m4w, n, h0, q, {})
        emit_tail_a(pending)
        emit_tail_b(pending)
        emit_tail_c(pending)


def _compile():
    if "nc" in _COMPILED:
        return _COMPILED["nc"]
    nc = bacc.Bacc("TRN2", target_bir_lowering=False, debug=False,
                   num_devices=N_CORES)
    with tile.TileContext(nc) as tc:
        _emit_kernel(tc)
    nc.compile()
    _COMPILED["nc"] = nc
    return nc


def kernel(x, conv_weight, conv_bias):
    x = np.asarray(x, dtype=np.float32)
    conv_weight = np.asarray(conv_weight, dtype=np.float32)
    conv_bias = np.asarray(conv_bias, dtype=np.float32)

    xp = np.ascontiguousarray(
        x.transpose(0, 2, 1, 3, 4)).astype(ml_dtypes.bfloat16)  # [N,D,C,H,W]
    w_sb = _build_weight_blocks(conv_weight)
    bias_sb = _build_bias128(conv_bias)
    ones_sb = _build_ones()
    ones_bc_sb = _build_ones_bc()

    nc = _compile()
    in_maps = []
    for i in range(N_CORES):
        in_maps.append({
            "x": np.ascontiguousarray(xp[NB * i:NB * (i + 1)]),
            "w": w_sb,
            "bias": bias_sb,
            "ones": ones_sb,
            "ones_bc": ones_bc_sb,
        })
    res = bass_utils.run_bass_kernel_spmd(
        nc, in_maps, core_ids=list(range(N_CORES)),
        trace=bool(int(os.environ.get("KERNEL_TRACE", "0"))),
    )
    _COMPILED["last_results"] = res
    out = np.concatenate([res.results[i]["y"] for i in range(N_CORES)], axis=0)
    return out


if __name__ == "__main__":
    _compile()
    print("build OK")


# revision 24
# speedup vs baseline: 1.0904x; 1.0549x over previous
"""Trainium2 Bass kernel for: Conv3d(3->16, k=3x3x3, VALID) + bias -> min over
depth -> softmax over channels.

Input  x: (16, 3, 32, 128, 128) f32   [N, C_in, D, H, W]
Weight w: (16, 3, 3, 3, 3) f32        [C_out, C_in, kD, kH, kW]
Bias   b: (16,) f32
Output  : (16, 16, 126, 126) f32      [N, C_out, H_out, W_out]

Data-parallel over batch: 2 batches per core x 8 cores. Per core:

  - x stored per (batch, h-half) as one [128, 8768] bf16 tile: strip r
    (partition quadrant 32r) holds 30 rows = (10 input depths 8r..8r+9) x
    (ci 3); free dim = local (h, w) flattened (66 or 64 h-rows + pad).
  - Conv as 4 row-packed matmuls per (chunk, khw): tile r = [K<=30, M=128,
    N=512] at tile_position (32r, 0); M = 8 local douts x 16 co; 9
    accumulating MMs over (kh,kw) with free-dim-shifted rhs (stride-1 conv
    == same spatial layout + offset koff). Weight col block[(dl,ci),
    (dll,co)] = w[co,ci,dl-dll,kh,kw]. PSUM supertile [128, 4*512]:
    bank r = strip r's 8 douts.
  - Strip 3 douts 30,31 don't exist: their weight cols are 0 except a
    BIG=32768 entry at khw=0 against a constant-1.0 rhs row, so those psum
    lanes hold +32768 and never win the depth-min.
  - Depth-min: one DVE tensor_reduce(min) over the 4 banks -> [128, 512],
    then a 128->64->32->16 partition tree-min on GpSimd into a [128, 512]
    collector (8 chunks x 16 co).
  - Softmax over co per 8-chunk group: ACT exp (bias fused; min(y)+b ==
    min(y+b)), PE ones-matmul for co-sums, DVE reciprocal, PE broadcast
    matmul, DVE multiply, DMA out valid rows.
"""

import os
import sys

sys.path.insert(0, "/opt/trn_rl_repo")

import numpy as np
import ml_dtypes

import concourse.bass as bass
import concourse.bacc as bacc
import concourse.tile as tile
import concourse.mybir as mybir
import concourse.bass_isa as bass_isa
from concourse import bass_utils

F32 = mybir.dt.float32
BF16 = mybir.dt.bfloat16

N_CORES = 8
NB = 2           # batches per core
CI = 3
D = 32
H = 128
W = 128
CO = 16
CHUNK = 512
HOUT = 126
WOUT = 126
PAD = 320
QF = 66 * W + PAD  # quad tile free size (worst case hh=0)
BIG = 32768.0

_COMPILED = {}


def _strip_depths(r):
    return 10 if r < 3 else 8


def _kr(r, khw):
    if r < 3:
        return 30
    return 25 if khw == 0 else 24


def _build_weight_blocks(conv_weight):
    """[128, 9*128]: strip r rows 32r+(3*dl+ci); col khw*128 + dll*16 + co
    = w[co, ci, dl-dll, kh, kw] (0 outside kd range / dout>=30). Row 120
    (strip 3 local 24) carries BIG at khw=0 for the dout 30/31 lanes."""
    wb = np.zeros((128, 9 * 128), dtype=np.float32)
    for r in range(4):
        for dl in range(_strip_depths(r)):
            for ci in range(CI):
                row = 32 * r + 3 * dl + ci
                for khw in range(9):
                    kh, kw = khw // 3, khw % 3
                    for dll in range(8):
                        kd = dl - dll
                        if 8 * r + dll < 30 and 0 <= kd <= 2:
                            wb[row, khw * 128 + dll * 16:
                               khw * 128 + dll * 16 + 16] = \
                                conv_weight[:, ci, kd, kh, kw]
    for dll in (6, 7):
        wb[120, dll * 16:dll * 16 + 16] = BIG
    return wb.astype(ml_dtypes.bfloat16)


def _build_ones():
    """[128, 8]: col j sums partitions {16j + co : co}."""
    ones = np.zeros((128, 8), dtype=np.float32)
    for p in range(128):
        ones[p, p // 16] = 1.0
    return ones


def _build_ones_bc():
    """[8, 128]: transpose -- broadcasts row j over its 16-partition group."""
    return np.ascontiguousarray(_build_ones().T)


def _build_bias128(conv_bias):
    """[128, 1]: partition 16j + co -> bias[co]."""
    b = np.zeros((128, 1), dtype=np.float32)
    for p in range(128):
        b[p, 0] = conv_bias[p % 16]
    return b


def _emit_kernel(tc):
    nc = tc.nc
    x_ap = nc.dram_tensor("x", [NB, D, CI, H, W], BF16,
                          kind="ExternalInput").ap()
    w_ap = nc.dram_tensor("w", [128, 9 * 128], BF16,
                          kind="ExternalInput").ap()
    bias_ap = nc.dram_tensor("bias", [128, 1], F32, kind="ExternalInput").ap()
    ones_ap = nc.dram_tensor("ones", [128, 8], F32, kind="ExternalInput").ap()
    ones_bc_ap = nc.dram_tensor("ones_bc", [8, 128], F32,
                                kind="ExternalInput").ap()
    y_ap = nc.dram_tensor("y", [NB, CO, HOUT, WOUT], F32,
                          kind="ExternalOutput").ap()

    from contextlib import ExitStack

    with ExitStack() as ctx:
        const_pool = ctx.enter_context(tc.tile_pool(name="const", bufs=1))
        in_pool = ctx.enter_context(tc.tile_pool(name="in", bufs=2))
        m4w_pool = ctx.enter_context(tc.tile_pool(name="m4w", bufs=2))
        sm_pool = ctx.enter_context(tc.tile_pool(name="sm", bufs=2))
        psum_pool = ctx.enter_context(tc.tile_pool(name="ps", bufs=2,
                                                   space="PSUM"))

        w_sb = const_pool.tile([128, 9 * 128], BF16, tag="w")
        nc.sync.dma_start(w_sb[:, :], w_ap[:, :])
        bias_sb = const_pool.tile([128, 1], F32, tag="bias")
        nc.sync.dma_start(bias_sb[:, :], bias_ap[:, :])
        ones_sb = const_pool.tile([128, 8], F32, tag="ones")
        nc.sync.dma_start(ones_sb[:, :], ones_ap[:, :])
        ones_bc_sb = const_pool.tile([8, 128], F32, tag="onesbc")
        nc.sync.dma_start(ones_bc_sb[:, :], ones_bc_ap[:, :])

        def emit_tail_a(t):
            m4w_t, n_t, h0_t, q_t, st = t
            # fold the 8 dout_local groups (partition tree-min); engine TTs
            # need equal SBUF partition bases, so shift the upper half down
            # via SBUF-to-SBUF DMA at each level.
            sh = sm_pool.tile([64, 8 * CHUNK], BF16, tag="sh")
            nc.sync.dma_start(sh[0:64, :], m4w_t[64:128, :])
            nc.vector.tensor_tensor(
                out=m4w_t[0:64, :], in0=m4w_t[0:64, :], in1=sh[0:64, :],
                op=mybir.AluOpType.min)
            nc.sync.dma_start(sh[0:32, :], m4w_t[32:64, :])
            nc.vector.tensor_tensor(
                out=m4w_t[0:32, :], in0=m4w_t[0:32, :], in1=sh[0:32, :],
                op=mybir.AluOpType.min)
            nc.sync.dma_start(sh[0:16, :], m4w_t[16:32, :])
            nc.vector.tensor_tensor(
                out=m4w_t[0:16, :], in0=m4w_t[0:16, :], in1=sh[0:16, :],
                op=mybir.AluOpType.min)
            # spread mins to the (chunk x co) softmax layout
            coll = sm_pool.tile([128, CHUNK], BF16, tag="coll")
            for j in range(8):
                nc.sync.dma_start(coll[16 * j:16 * j + 16, :],
                                  m4w_t[0:16, j * CHUNK:(j + 1) * CHUNK])
            expt = sm_pool.tile([128, CHUNK], F32, tag="exp")
            nc.scalar.activation(
                expt[:, :], coll[:, :],
                mybir.ActivationFunctionType.Exp,
                bias=bias_sb[:, :], scale=1.0)
            st["expt"] = expt

        def emit_tail_b(t):
            m4w_t, n_t, h0_t, q_t, st = t
            # esum and rb share one psum tile: esum lives in partitions 0..8,
            # is consumed by the reciprocal, then rb's matmul overwrites.
            T = psum_pool.tile([128, CHUNK], F32, tag="big")
            nc.tensor.matmul(T[0:8, :], lhsT=ones_sb[:, :],
                             rhs=st["expt"][:, :], start=True, stop=True)
            rec = sm_pool.tile([8, CHUNK], F32, tag="rec")
            nc.vector.reciprocal_approx_fast(rec[:, :], T[0:8, :])
            st["rec"] = rec
            st["T"] = T

        def emit_tail_c(t):
            m4w_t, n_t, h0_t, q_t, st = t
            T = st["T"]
            nc.tensor.matmul(T[:, :], lhsT=ones_bc_sb[:, :],
                             rhs=st["rec"][:, :], start=True, stop=True)
            soft = sm_pool.tile([128, CHUNK], F32, tag="soft")
            nc.vector.tensor_tensor(
                out=soft[:, :], in0=T[:, :], in1=st["expt"][:, :],
                op=mybir.AluOpType.mult)
            for j in range(8):
                hc = h0_t + 4 * (8 * q_t + j)
                ni = min(4, HOUT - hc)
                if ni <= 0:
                    continue
                srcj = soft[16 * j:16 * j + 16, :].rearrange(
                    "co (i w) -> co i w", i=4)[:, 0:ni, 0:WOUT]
                nc.sync.dma_start(y_ap[n_t, :, hc:hc + ni, :], srcj)

        pending = None
        items = [(n, hh) for n in range(NB) for hh in range(2)]
        quads = {}

        def prep_quad(k):
            n_k, hh_k = items[k]
            h0_k = 64 * hh_k
            hrows = 66 if hh_k == 0 else 64
            quad_k = in_pool.tile([128, QF], BF16, tag="quad")
            nc.gpsimd.memset(quad_k[:, hrows * W:QF], 0.0)
            # rows 96..127 <- 1.0; strip-3 DMA overwrites 96..119, leaving
            # the constant-1.0 row 120 for the BIG sentinel matmul column.
            nc.gpsimd.memset(quad_k[96:128, 0:hrows * W], 1.0)
            for r in range(4):
                nd = _strip_depths(r)
                srcr = x_ap[n_k, 8 * r:8 * r + nd, :,
                            h0_k:h0_k + hrows, :].rearrange(
                    "d c h w -> (d c) (h w)")
                nc.sync.dma_start(
                    quad_k[32 * r:32 * r + 3 * nd, 0:hrows * W], srcr)
            quads[k] = quad_k

        prep_quad(0)
        for k, (n, hh) in enumerate(items):
            h0 = 64 * hh
            quad = quads.pop(k)
            if True:
                for q in range(2):
                    m4w = m4w_pool.tile([128, 8 * CHUNK], BF16, tag="m4w")
                    for j in range(8):
                        m = 8 * q + j
                        s0 = CHUNK * m
                        ps = psum_pool.tile([128, 4 * CHUNK], F32, tag="big")
                        for khw in range(9):
                            kh, kw = khw // 3, khw % 3
                            koff = kh * W + kw
                            for r in range(4):
                                kr = _kr(r, khw)
                                nc.tensor.matmul(
                                    ps[:, r * CHUNK:(r + 1) * CHUNK],
                                    lhsT=w_sb[32 * r:32 * r + kr,
                                              khw * 128:(khw + 1) * 128],
                                    rhs=quad[32 * r:32 * r + kr,
                                             s0 + koff:s0 + koff + CHUNK],
                                    start=(khw == 0),
                                    stop=(khw == 8),
                                    tile_position=(32 * r, 0),
                                    skip_group_check=True,
                                )
                        nc.vector.tensor_reduce(
                            out=m4w[:, j * CHUNK:(j + 1) * CHUNK],
                            in_=ps[:, :].rearrange("p (r s) -> p s r",
                                                   s=CHUNK),
                            axis=mybir.AxisListType.X,
                            op=mybir.AluOpType.min,
                        )
                        # software-pipelined: previous group's tail ops
                        # land between this group's chunks so every engine
                        # FIFO sees them with ready inputs and the psum
                        # rotation never blocks on a long chain.
                        if pending is not None:
                            if j == 0:
                                emit_tail_a(pending)
                            elif j == 6:
                                emit_tail_b(pending)
                            elif j == 7:
                                emit_tail_c(pending)
                                pending = None
                        if q == 0 and j == 3 and k + 1 < len(items):
                            prep_quad(k + 1)
                    pending = (m4w, n, h0, q, {})
        emit_tail_a(pending)
        emit_tail_b(pending)
        emit_tail_c(pending)


def _compile():
    if "nc" in _COMPILED:
        return _COMPILED["nc"]
    nc = bacc.Bacc("TRN2", target_bir_lowering=False, debug=False,
                   num_devices=N_CORES)
    with tile.TileContext(nc) as tc:
        _emit_kernel(tc)
    nc.compile()
    _COMPILED["nc"] = nc
    return nc


def kernel(x, conv_weight, conv_bias):
    x = np.asarray(x, dtype=np.float32)
    conv_weight = np.asarray(conv_weight, dtype=np.float32)
    conv_bias = np.asarray(conv_bias, dtype=np.float32)

    xp = np.ascontiguousarray(
        x.transpose(0, 2, 1, 3, 4)).astype(ml_dtypes.bfloat16)  # [N,D,C,H,W]
    w_sb = _build_weight_blocks(conv_weight)
    bias_sb = _build_bias128(conv_bias)
    ones_sb = _build_ones()
    ones_bc_sb = _build_ones_bc()

    nc = _compile()
    in_maps = []
    for i in range(N_CORES):
        in_maps.append({
            "x": np.ascontiguousarray(xp[NB * i:NB * (i + 1)]),
            "w": w_sb,
            "bias": bias_sb,
            "ones": ones_sb,
            "ones_bc": ones_bc_sb,
        })
    res = bass_utils.run_bass_kernel_spmd(
        nc, in_maps, core_ids=list(range(N_CORES)),
        trace=bool(int(os.environ.get("KERNEL_TRACE", "0"))),
    )
    _COMPILED["last_results"] = res
    out = np.concatenate([res.results[i]["y"] for i in range(N_CORES)], axis=0)
    return out


if __name__ == "__main__":
    _compile()
    print("build OK")
